# revision 1
# baseline (speedup 1.0000x reference)
"""Distributed GQA attention kernel for Trainium2 (8 NeuronCores).

Module: B=4, S=2048, H=576, 9 Q heads / 3 KV heads, HD=64, RoPE, causal
softmax, output projection.

Sharding: core c handles batch c//2 and four 256-row query blocks
({0,3,4,7} for even c, {1,2,5,6} for odd c) -- causal work is balanced at
18 key-tile units per core. Every core computes its batch's full K/V
projection locally (duplicated across the 2 cores of a batch; cheaper
than an all-gather). One SPMD graph for all 8 cores: per-slot key-tile
extents are padded to [4,8,12,16] and the causal mask is applied from
per-core mask DATA on the last 4 key-tiles of each slot.

Everything lives transposed ([dim, seq]) so scoresT[k,q] chains
QK -> exp -> PV -> Wo with no on-chip transposes. Softmax skips the max
subtraction (scores are O(1) here); row-sums ride along as a 65th output
row of the PV matmul via a ones-column appended to V. Matmuls run in
float32r (full PE rate at free-dim >= 256).
"""

import sys

if "/opt/trn_rl_repo" not in sys.path:
    sys.path.insert(0, "/opt/trn_rl_repo")

import numpy as np

B, S, H = 4, 2048, 576
NH, NKV, HD = 9, 3, 64
GROUPS = NH // NKV  # 3 q heads per kv head
BLK = 256           # query block rows
NBLK = S // BLK     # 8
KT = 128            # key tile rows
EXT = [4, 8, 12, 16]  # padded key-tile extent per block slot
NMASK = 16          # total masked key-tiles per core (= sum of last-4 per slot)
BLOCKS_EVEN = [0, 3, 4, 7]
BLOCKS_ODD = [1, 2, 5, 6]
HK = [128, 128, 128, 128, 64]  # contraction tiles over H=576

_CACHED = {}


USE_BF16 = False


def _build(reps=1):
    from concourse import bacc, bass, mybir, tile

    f32 = mybir.dt.float32
    f32r = mybir.dt.bfloat16 if USE_BF16 else mybir.dt.float32r
    AF = mybir.ActivationFunctionType
    ALU = mybir.AluOpType

    nc = bacc.Bacc("TRN2", target_bir_lowering=False, debug=False)

    # ---- per-core inputs ----
    xT = nc.dram_tensor("xT", [H, S], f32r, kind="ExternalInput")
    xTq = nc.dram_tensor("xTq", [H, 4 * BLK], f32r, kind="ExternalInput")
    Wq = nc.dram_tensor("Wq", [H, NH * HD], f32r, kind="ExternalInput")
    Wk = nc.dram_tensor("Wk", [H, NKV * HD], f32r, kind="ExternalInput")
    Wvp = nc.dram_tensor("Wvp", [H, 256], f32r, kind="ExternalInput")  # Wv zero-padded
    Wo = nc.dram_tensor("Wo", [NH * HD, H], f32r, kind="ExternalInput")
    P = nc.dram_tensor("P", [HD, HD], f32r, kind="ExternalInput")  # rotate_half perm
    cosk = nc.dram_tensor("cosk", [HD, S], f32r, kind="ExternalInput")
    sink = nc.dram_tensor("sink", [HD, S], f32r, kind="ExternalInput")
    cosq = nc.dram_tensor("cosq", [HD, 4 * BLK], f32r, kind="ExternalInput")  # /8
    sinq = nc.dram_tensor("sinq", [HD, 4 * BLK], f32r, kind="ExternalInput")  # /8
    maskst = nc.dram_tensor("maskst", [NMASK, KT, BLK], f32r, kind="ExternalInput")
    ones16 = nc.dram_tensor("ones16", [128, 16], f32r, kind="ExternalInput")
    out = nc.dram_tensor("out", [4 * BLK, H], f32, kind="ExternalOutput")

    with tile.TileContext(nc) as tc:
        with (
            tc.tile_pool(name="consts", bufs=1) as cp,
            tc.tile_pool(name="xstream", bufs=2) as xsp,
            tc.tile_pool(name="kvres", bufs=1) as kvres,
            tc.tile_pool(name="qtp", bufs=1) as qtp,
            tc.tile_pool(name="work", bufs=2) as wp,
            tc.tile_pool(name="expp", bufs=4) as expp,
            tc.tile_pool(name="mskp", bufs=1) as mskp,
            tc.tile_pool(name="ctp", bufs=1) as ctp,
            tc.tile_pool(name="outp", bufs=2) as outp,
            tc.tile_pool(name="ps1", bufs=2, space="PSUM") as ps1,
            tc.tile_pool(name="scp", bufs=2, space="PSUM") as scp,
            tc.tile_pool(name="psA", bufs=1, space="PSUM") as psA,
        ):
            # ---- load constants ----
            def load_w(dram, cols, defer=False):
                tiles = []
                r0 = 0
                for hk in HK:
                    t = cp.tile([hk, cols], f32r, tag=f"w{dram.name}{r0}", name=f"w{dram.name}{r0}")
                    if not defer:
                        nc.sync.dma_start(t[:], dram.ap()[r0 : r0 + hk, :])
                    tiles.append(t)
                    r0 += hk
                return tiles

            Wk_sb = load_w(Wk, NKV * HD)
            Wvp_sb = load_w(Wvp, 256)
            Wq_sb = load_w(Wq, NH * HD)
            Wo_sb = load_w(Wo, H)
            P_sb = cp.tile([HD, HD], f32r, tag="P")
            nc.sync.dma_start(P_sb[:], P.ap())
            cosq_sb = cp.tile([HD, 4 * BLK], f32r, tag="cosq")
            sinq_sb = cp.tile([HD, 4 * BLK], f32r, tag="sinq")
            for t, d in ((cosq_sb, cosq), (sinq_sb, sinq)):
                nc.sync.dma_start(t[:], d.ap())
            ones_sb = cp.tile([128, 16], f32r, tag="ones")
            nc.sync.dma_start(ones_sb[:], ones16.ap())
            # trigger the exp ACT-table load during the startup DMA wait
            warm = cp.tile([1, 1], f32, tag="warm")
            nc.scalar.activation(warm[:], ones_sb[0:1, 0:1], AF.Exp)

            def one_pass():
                # ---- K/V projection, streaming xT in 512-token chunks ----
                # (replicated `reps` times for slope timing; reps=1 in production)
                kTp = [
                    [kvres.tile([HD, 512], f32r, tag=f"kT{g}c{ch}", name=f"kTp{g}c{ch}") for ch in range(4)]
                    for g in range(NKV)
                ]
                v_aug = [
                    [kvres.tile([128, 4 * 65], f32r, tag=f"vaug{g}c{ch}", name=f"vaug{g}c{ch}") for ch in range(4)]
                    for g in range(NKV)
                ]
                for ch in range(4):
                    c0 = ch * 512
                    xch = []
                    r0 = 0
                    for kt, hk in enumerate(HK):
                        t = xsp.tile([hk, 512], f32r, tag=f"xch{kt}", name=f"xch{kt}")
                        nc.sync.dma_start(t[:], xT.ap()[r0 : r0 + hk, c0 : c0 + 512])
                        xch.append(t)
                        r0 += hk
                    cosk_ch = xsp.tile([HD, 512], f32r, tag="coskch", name="cosk_ch")
                    nc.sync.dma_start(cosk_ch[:], cosk.ap()[:, c0 : c0 + 512])
                    sink_ch = xsp.tile([HD, 512], f32r, tag="sinkch", name="sink_ch")
                    nc.sync.dma_start(sink_ch[:], sink.ap()[:, c0 : c0 + 512])
                    for g in range(NKV):
                        kps = ps1.tile([HD, 512], f32, tag="ps1")
                        for kt in range(5):
                            nc.tensor.matmul(
                                kps[:],
                                Wk_sb[kt][:, g * HD : (g + 1) * HD],
                                xch[kt][:],
                                start=(kt == 0),
                                stop=(kt == 4),
                            )
                        kraw = wp.tile([HD, 512], f32r, tag="kraw")
                        nc.vector.tensor_copy(kraw[:], kps[:])
                        rps = ps1.tile([HD, 512], f32, tag="ps1")
                        nc.tensor.matmul(rps[:], P_sb[:], kraw[:], start=True, stop=True)
                        t1 = wp.tile([HD, 512], f32r, tag="t1")
                        nc.vector.tensor_tensor(t1[:], kraw[:], cosk_ch[:], ALU.mult)
                        t2 = wp.tile([HD, 512], f32r, tag="t2")
                        nc.vector.tensor_tensor(t2[:], rps[:], sink_ch[:], ALU.mult)
                        nc.vector.tensor_tensor(kTp[g][ch][:], t1[:], t2[:], ALU.add)
                    for st4 in range(4):
                        st = ch * 4 + st4
                        vps = ps1.tile([128, 256], f32, tag="ps1")
                        for kt in range(5):
                            nc.tensor.matmul(
                                vps[:],
                                xch[kt][:, st4 * 128 : (st4 + 1) * 128],
                                Wvp_sb[kt][:],
                                start=(kt == 0),
                                stop=(kt == 4),
                            )
                        for g in range(NKV):
                            nc.vector.tensor_copy(
                                v_aug[g][ch][:, st4 * 65 : st4 * 65 + 64],
                                vps[:, g * HD : (g + 1) * HD],
                            )
                for g in range(NKV):
                    for ch in range(4):
                        dst = v_aug[g][ch][:].rearrange("p (n c) -> p n c", c=65)[:, :, 64:65]
                        nc.vector.tensor_copy(dst, ones_sb[:, 0:4].unsqueeze(2))

                # ---- Q proj + RoPE for all 4 blocks at once ----
                xq = []
                r0 = 0
                for kt, hk in enumerate(HK):
                    t = qtp.tile([hk, 4 * BLK], f32r, tag=f"xq{kt}", name=f"xq{kt}")
                    nc.sync.dma_start(t[:], xTq.ap()[r0 : r0 + hk, :])
                    xq.append(t)
                    r0 += hk
                qTall = qtp.tile([HD, NH * 4 * BLK], f32r, tag="qTall", name="qTall")
                # layout: [64, h*1024 + blk*256]
                for h in range(NH):
                    for cc in range(2):  # two 512-col chunks of the 1024 q cols
                        qps = ps1.tile([HD, 512], f32, tag="ps1")
                        for kt in range(5):
                            nc.tensor.matmul(
                                qps[:],
                                Wq_sb[kt][:, h * HD : (h + 1) * HD],
                                xq[kt][:, cc * 512 : (cc + 1) * 512],
                                start=(kt == 0),
                                stop=(kt == 4),
                            )
                        qraw = wp.tile([HD, 512], f32r, tag="kraw")
                        nc.vector.tensor_copy(qraw[:], qps[:])
                        rps = ps1.tile([HD, 512], f32, tag="ps1")
                        nc.tensor.matmul(rps[:], P_sb[:], qraw[:], start=True, stop=True)
                        tq1 = wp.tile([HD, 512], f32r, tag="t1")
                        nc.vector.tensor_tensor(
                            tq1[:], qraw[:], cosq_sb[:, cc * 512 : (cc + 1) * 512], ALU.mult
                        )
                        tq2 = wp.tile([HD, 512], f32r, tag="t2")
                        nc.vector.tensor_tensor(
                            tq2[:], rps[:], sinq_sb[:, cc * 512 : (cc + 1) * 512], ALU.mult
                        )
                        nc.vector.tensor_tensor(
                            qTall[:, h * 1024 + cc * 512 : h * 1024 + (cc + 1) * 512],
                            tq1[:],
                            tq2[:],
                            ALU.add,
                        )

                for j in range(4):
                    q0 = j * BLK
                    qv = qTall[:].rearrange("p (h j c) -> p h j c", j=4, c=BLK)
                    # concat tiles for Wo lhsT: heads 2t (rows 0:64), 2t+1 (rows 64:128)
                    cts = [ctp.tile([128, BLK], f32r, tag=f"ct{t}", name=f"ct{t}") for t in range(4)]
                    cts.append(ctp.tile([HD, BLK], f32r, tag="ct4", name="ct4"))

                    ext = EXT[j]
                    mts = {}
                    for off in range(4):
                        kcm = ext - 4 + off
                        mt = mskp.tile([KT, BLK], f32r, tag=f"msk{off}", name=f"msk{off}")
                        nc.sync.dma_start(mt[:], maskst.ap()[kcm, :, :])
                        mts[kcm] = mt
                    for g in range(NKV):
                        h0 = 3 * g
                        accp = psA.tile([65, 512], f32, tag="accp", name="accp")
                        accs1 = psA.tile([65, BLK], f32, tag="accs", name="accs")
                        for kc in range(ext):
                            masked = kc >= ext - 4
                            if masked:
                                mt = mts[kc]
                            sps = scp.tile([KT, 3 * BLK], f32, tag="sc")
                            nc.tensor.matmul(
                                sps[:, 0:512],
                                kTp[g][kc // 4][:, (kc % 4) * KT : (kc % 4 + 1) * KT],
                                qv[:, h0 : h0 + 2, j, :],
                                start=True,
                                stop=True,
                            )
                            nc.tensor.matmul(
                                sps[:, 512:768],
                                kTp[g][kc // 4][:, (kc % 4) * KT : (kc % 4 + 1) * KT],
                                qv[:, h0 + 2 : h0 + 3, j, :],
                                start=True,
                                stop=True,
                            )
                            esb = expp.tile([KT, 3 * BLK], f32r, tag="exp")
                            nc.scalar.activation(esb[:], sps[:], AF.Exp)
                            if masked:
                                for i in range(3):
                                    sl = esb[:, i * BLK : (i + 1) * BLK]
                                    nc.gpsimd.tensor_tensor(sl, sl, mt[:], ALU.mult)
                            nc.tensor.matmul(
                                accp[:],
                                v_aug[g][kc // 4][:, (kc % 4) * 65 : (kc % 4) * 65 + 65],
                                esb[:, 0:512],
                                start=(kc == 0),
                                stop=(kc == ext - 1),
                            )
                            nc.tensor.matmul(
                                accs1[:],
                                v_aug[g][kc // 4][:, (kc % 4) * 65 : (kc % 4) * 65 + 65],
                                esb[:, 512:768],
                                start=(kc == 0),
                                stop=(kc == ext - 1),
                            )
                        for acc, width, heads in (
                            (accp, 512, (h0, h0 + 1)),
                            (accs1, 256, (h0 + 2,)),
                        ):
                            rec = wp.tile([128, 512], f32, tag="rec")
                            nc.vector.reciprocal(rec[64:65, 0:width], acc[64:65, 0:width])
                            nc.sync.dma_start(rec[0:1, 0:width], rec[64:65, 0:width])
                            bc = wp.tile([HD, 512], f32, tag="bc")
                            nc.gpsimd.partition_broadcast(bc[:, 0:width], rec[0:1, 0:width])
                            for i, h in enumerate(heads):
                                c0 = i * BLK
                                t, lo = divmod(h, 2)
                                if lo == 0:
                                    nc.vector.tensor_tensor(
                                        cts[t][0:HD, :],
                                        acc[0:HD, c0 : c0 + BLK],
                                        bc[:, c0 : c0 + BLK],
                                        ALU.mult,
                                    )
                                else:
                                    stg = wp.tile([HD, BLK], f32r, tag="stg")
                                    nc.vector.tensor_tensor(
                                        stg[:],
                                        acc[0:HD, c0 : c0 + BLK],
                                        bc[:, c0 : c0 + BLK],
                                        ALU.mult,
                                    )
                                    nc.sync.dma_start(cts[t][HD:128, :], stg[:])

                    # out projection: out[q, :] = sum_t cts[t][:, q].T @ Wo_sb[t]
                    for half in range(2):
                        h0 = half * 128
                        pa = ps1.tile([128, 512], f32, tag="ps1")
                        pb = ps1.tile([128, 64], f32, tag="ps1")
                        for t in range(5):
                            lhsT = cts[t][:, h0 : h0 + 128]
                            nc.tensor.matmul(
                                pa[:], lhsT, Wo_sb[t][:, 0:512], start=(t == 0), stop=(t == 4)
                            )
                            nc.tensor.matmul(
                                pb[:], lhsT, Wo_sb[t][:, 512:576], start=(t == 0), stop=(t == 4)
                            )
                        osb = outp.tile([128, H], f32, tag="osb")
                        nc.vector.tensor_copy(osb[:, 0:512], pa[:])
                        nc.vector.tensor_copy(osb[:, 512:576], pb[:])
                        nc.sync.dma_start(out.ap()[q0 + h0 : q0 + h0 + 128, :], osb[:])

            for _rep in range(reps):
                one_pass()

    nc.compile()
    return nc


def _get_nc(reps=1):
    key = f"nc{reps}"
    if key not in _CACHED:
        _CACHED[key] = _build(reps=reps)
    return _CACHED[key]


def _make_in_maps(x, cos, sin, mask, Wq, Wk, Wv, Wo):
    f4 = np.float32
    if USE_BF16:
        import ml_dtypes

        dtc = ml_dtypes.bfloat16
    else:
        dtc = np.float32
    Wvp = np.zeros((H, 256), f4)
    Wvp[:, : NKV * HD] = Wv
    P = np.zeros((HD, HD), f4)
    half = HD // 2
    for m in range(half):
        P[m + half, m] = -1.0
    for m in range(half, HD):
        P[m - half, m] = 1.0
    cosT = np.ascontiguousarray(cos.T.astype(f4))  # [64, S]
    sinT = np.ascontiguousarray(sin.T.astype(f4))
    scale = np.float32(1.0 / np.sqrt(HD))
    maskT_full = np.ascontiguousarray(mask[0, 0].T.astype(f4))  # [k, q]
    ones16 = np.ones((128, 16), f4)

    in_maps = []
    for c in range(8):
        b = c // 2
        blocks = BLOCKS_EVEN if c % 2 == 0 else BLOCKS_ODD
        xb = x[b]  # [S, H]
        xTc = np.ascontiguousarray(xb.T.astype(f4))  # [H, S]
        qcols = np.concatenate([xTc[:, blk * BLK : (blk + 1) * BLK] for blk in blocks], axis=1)
        cosq = np.concatenate(
            [cosT[:, blk * BLK : (blk + 1) * BLK] for blk in blocks], axis=1
        ) * scale
        sinq = np.concatenate(
            [sinT[:, blk * BLK : (blk + 1) * BLK] for blk in blocks], axis=1
        ) * scale
        maskstk = np.empty((NMASK, KT, BLK), f4)
        for j, blk in enumerate(blocks):
            for off in range(4):
                kc = 4 * j + off
                sl = maskT_full[kc * KT : (kc + 1) * KT, blk * BLK : (blk + 1) * BLK]
                maskstk[kc] = (sl > -1.0).astype(f4)
        in_maps.append(
            {
                "xT": xTc.astype(dtc),
                "xTq": np.ascontiguousarray(qcols).astype(dtc),
                "Wq": Wq.astype(f4).astype(dtc),
                "Wk": Wk.astype(f4).astype(dtc),
                "Wvp": Wvp.astype(dtc),
                "Wo": Wo.astype(f4).astype(dtc),
                "P": P.astype(dtc),
                "cosk": cosT.astype(dtc),
                "sink": sinT.astype(dtc),
                "cosq": np.ascontiguousarray(cosq).astype(dtc),
                "sinq": np.ascontiguousarray(sinq).astype(dtc),
                "maskst": maskstk.astype(dtc),
                "ones16": ones16.astype(dtc),
            }
        )
    return in_maps


def kernel(x, cos, sin, mask, Wq, Wk, Wv, Wo, _trace=False, _trace_kwargs=None):
    from concourse import bass_utils

    x = np.asarray(x)
    in_maps = _make_in_maps(
        np.asarray(x), np.asarray(cos), np.asarray(sin), np.asarray(mask),
        np.asarray(Wq), np.asarray(Wk), np.asarray(Wv), np.asarray(Wo),
    )
    nc = _get_nc()
    kw = {}
    if _trace:
        kw["trace"] = True
        if _trace_kwargs:
            kw.update(_trace_kwargs)
    res = bass_utils.run_bass_kernel_spmd(nc, in_maps, core_ids=list(range(8)), **kw)
    out = np.empty((B, S, H), np.float32)
    for c in range(8):
        b = c // 2
        blocks = BLOCKS_EVEN if c % 2 == 0 else BLOCKS_ODD
        o = res.results[c]["out"]  # [1024, 576]
        for j, blk in enumerate(blocks):
            out[b, blk * BLK : (blk + 1) * BLK, :] = o[j * BLK : (j + 1) * BLK, :]
    if _trace:
        _CACHED["last_result"] = res
    return out

